# revision 3
# baseline (speedup 1.0000x reference)
"""Distributed full-hidden attention on 8 trn2 NeuronCores.

Math (per reference):
  q = x @ Wq.T + bq ; k, v likewise          [B, S, H]
  scores = q @ k.T / sqrt(64)                [B, S, S]  (full hidden dim)
  out = softmax(scores) @ v @ Wo.T + bo      [B, S, H]

Sharding: sequence-parallel. Core r owns tokens x[:, r*512:(r+1)*512, :].
Each core projects its own q/k/v, AllGathers K^T and V (bf16) so every
core holds the full key/value set, computes attention for its local
queries (softmax without max-subtraction — |scores| <= ~34, safe in
fp32), applies the output projection, and writes its token shard.
Host concatenates shards.

Precision: projections and out-proj run on the PE in fp32r (full rate,
~1.5e-4 matmul rel err); the two big attention matmuls run in bf16.

Layouts per core (t = b*512 + s_local, 1024 local tokens):
  xT   [H, T]   fp32r  (host-transposed)
  qT,kT = W.T-swapped matmuls -> [o, t] tiles; v natural [t, o]
  scoresT[sk, tq] = kT.T @ qT ; exp on ACT (scale=1/8)
  d = ones.T @ exp (per-column sums broadcast over partitions)
  exp_norm = exp / d ; attnT[h, tq] = V.T-as-lhsT @ exp_norm
  out[t, o] = attnT.T @ WoT + bo
"""

import numpy as np
import ml_dtypes

import concourse.bass as bass
import concourse.mybir as mybir
import concourse.tile as tile
from concourse import bacc
from concourse.bass_utils import run_bass_kernel_spmd

N_CORES = 8
B, S, H = 2, 4096, 1024
S_LOC = S // N_CORES      # 512 tokens per batch per core
T = B * S_LOC             # 1024 local tokens
P = 128
HC = H // P               # 8 chunks of the hidden/output dim
NKC = S // P              # 32 key chunks of 128 per batch
F32 = mybir.dt.float32
F32R = mybir.dt.float32r
BF16 = mybir.dt.bfloat16
AF = mybir.ActivationFunctionType
SCALE = 1.0 / 8.0         # 1/sqrt(HEAD_DIM=64)

_CACHE = {}


def build_program():
    nc = bacc.Bacc(
        "TRN2",
        target_bir_lowering=False,
        debug=False,
        enable_asserts=False,
        num_devices=N_CORES,
    )

    xT_d = nc.dram_tensor("xT", [H, T], F32R, kind="ExternalInput").ap()
    wqT_d = nc.dram_tensor("wqT", [H, H], F32R, kind="ExternalInput").ap()
    wkT_d = nc.dram_tensor("wkT", [H, H], F32R, kind="ExternalInput").ap()
    wvT_d = nc.dram_tensor("wvT", [H, H], F32R, kind="ExternalInput").ap()
    woT_d = nc.dram_tensor("woT", [H, H], F32R, kind="ExternalInput").ap()
    bq_d = nc.dram_tensor("bq", [H], F32, kind="ExternalInput").ap()
    bk_d = nc.dram_tensor("bk", [H], F32, kind="ExternalInput").ap()
    bv_d = nc.dram_tensor("bv", [H], F32, kind="ExternalInput").ap()
    bo_d = nc.dram_tensor("bo", [H], F32, kind="ExternalInput").ap()
    out_d = nc.dram_tensor("out", [T, H], F32, kind="ExternalOutput").ap()

    with tile.TileContext(nc) as tc:
        with (
            tc.tile_pool(name="psum", bufs=8, space="PSUM") as pp,
            tc.tile_pool(name="dram", bufs=1, space="DRAM") as pdram,
            tc.tile_pool(name="misc", bufs=1) as pmisc,
            tc.tile_pool(name="qTp", bufs=1) as pqT,
        ):
            kb = pdram.tile([H, T], BF16, name="kb")
            vb = pdram.tile([T, H], BF16, name="vb")
            kag = pdram.tile([N_CORES * H, T], BF16, addr_space="Shared", name="kag")
            vag = pdram.tile([N_CORES * T, H], BF16, addr_space="Shared", name="vag")

            # --- small constants / biases -------------------------------
            ones1 = pmisc.tile([1, P], F32, name="ones1")
            nc.vector.memset(ones1[:], 1.0)
            ones128 = pmisc.tile([P, P], BF16, name="ones128")
            nc.vector.memset(ones128[:], 1.0)

            bqT = pmisc.tile([P, HC], F32, name="bqT")
            nc.sync.dma_start(bqT[:], bq_d.rearrange("(a p) -> p a", p=P))
            bkT = pmisc.tile([P, HC], F32, name="bkT")
            nc.sync.dma_start(bkT[:], bk_d.rearrange("(a p) -> p a", p=P))

            bv_row = pmisc.tile([1, H], F32, name="bv_row")
            nc.sync.dma_start(bv_row[:], bv_d.rearrange("(o h) -> o h", o=1))
            bo_row = pmisc.tile([1, H], F32, name="bo_row")
            nc.sync.dma_start(bo_row[:], bo_d.rearrange("(o h) -> o h", o=1))
            bv_bc = pmisc.tile([P, H], F32, name="bv_bc")
            bo_bc = pmisc.tile([P, H], F32, name="bo_bc")
            for src, dst in ((bv_row, bv_bc), (bo_row, bo_bc)):
                for c2 in range(2):
                    ps = pp.tile([P, 512], F32, name="ps_b", tag="bank")
                    nc.tensor.matmul(
                        ps[:], ones1[:], src[:, c2 * 512:(c2 + 1) * 512],
                        start=True, stop=True,
                    )
                    nc.scalar.activation(
                        dst[:, c2 * 512:(c2 + 1) * 512], ps[:], AF.Copy
                    )

            qT_sb = [pqT.tile([P, T], BF16, name=f"qT{h}") for h in range(HC)]

            # =========== phase A: projections ===========================
            with (
                tc.tile_pool(name="xTp", bufs=1) as pxT,
                tc.tile_pool(name="wp", bufs=9) as pw,
                tc.tile_pool(name="kvp", bufs=1) as pkv,
            ):
                xT_sb = []
                for h in range(HC):
                    t_ = pxT.tile([P, T], F32R, name=f"xT{h}")
                    nc.sync.dma_start(t_[:], xT_d[h * P:(h + 1) * P, :])
                    xT_sb.append(t_)

                kT_sb = [pkv.tile([P, T], BF16, name=f"kT{h}") for h in range(HC)]
                v_sb = [pkv.tile([P, H], BF16, name=f"v{h}") for h in range(HC)]

                def load_w(w_dram):
                    tiles = []
                    for h in range(HC):
                        w_t = pw.tile([P, H], F32R, name="w", tag="w")
                        nc.sync.dma_start(w_t[:], w_dram[h * P:(h + 1) * P, :])
                        tiles.append(w_t)
                    return tiles

                def proj_T(w_tiles, bias_sb, out_tiles):
                    # out[o, t] = W @ x.T   (o on partitions)
                    for tc2 in range(2):
                        for oc in range(HC):
                            ps = pp.tile([P, 512], F32, name="ps_p", tag="bank")
                            for h in range(HC):
                                nc.tensor.matmul(
                                    ps[:],
                                    w_tiles[h][:, oc * P:(oc + 1) * P],
                                    xT_sb[h][:, tc2 * 512:(tc2 + 1) * 512],
                                    start=(h == 0), stop=(h == HC - 1),
                                )
                            nc.scalar.activation(
                                out_tiles[oc][:, tc2 * 512:(tc2 + 1) * 512],
                                ps[:], AF.Identity, bias=bias_sb[:, oc:oc + 1],
                            )

                # K first so its AllGather launches earliest
                wk = load_w(wkT_d)
                proj_T(wk, bkT, kT_sb)
                for oc in range(HC):
                    nc.sync.dma_start(kb[oc * P:(oc + 1) * P, :], kT_sb[oc][:])
                nc.gpsimd.collective_compute(
                    "AllGather", mybir.AluOpType.bypass,
                    ins=[kb.opt()], outs=[kag.opt()],
                    replica_groups=[list(range(N_CORES))],
                )

                # V next (natural layout [t, o])
                wv = load_w(wvT_d)
                for ts in range(HC):
                    for oc2 in range(2):
                        ps = pp.tile([P, 512], F32, name="ps_v", tag="bank")
                        for h in range(HC):
                            nc.tensor.matmul(
                                ps[:],
                                xT_sb[h][:, ts * P:(ts + 1) * P],
                                wv[h][:, oc2 * 512:(oc2 + 1) * 512],
                                start=(h == 0), stop=(h == HC - 1),
                            )
                        nc.vector.tensor_add(
                            v_sb[ts][:, oc2 * 512:(oc2 + 1) * 512],
                            ps[:], bv_bc[:, oc2 * 512:(oc2 + 1) * 512],
                        )
                for ts in range(HC):
                    nc.sync.dma_start(vb[ts * P:(ts + 1) * P, :], v_sb[ts][:])
                nc.gpsimd.collective_compute(
                    "AllGather", mybir.AluOpType.bypass,
                    ins=[vb.opt()], outs=[vag.opt()],
                    replica_groups=[list(range(N_CORES))],
                )

                wq = load_w(wqT_d)
                proj_T(wq, bqT, qT_sb)

            # =========== phase B: attention + out-proj ==================
            with (
                tc.tile_pool(name="ktp", bufs=24) as pKT,
                tc.tile_pool(name="expp", bufs=32) as pexp,
                tc.tile_pool(name="vstp", bufs=8) as pV,
                tc.tile_pool(name="attp", bufs=1) as pattn,
                tc.tile_pool(name="wop", bufs=1) as pwo,
                tc.tile_pool(name="outp", bufs=4) as pout,
                tc.tile_pool(name="rdp", bufs=2) as prd,
            ):
                attnT = [pattn.tile([P, T], F32R, name=f"attnT{h}") for h in range(HC)]

                for b in range(B):
                    cb = slice(b * S_LOC, (b + 1) * S_LOC)
                    # ---- scoresT + exp + column sums ----
                    exp_tiles = []
                    dmat_ps = pp.tile([P, S_LOC], F32, name="dmat", tag="bank")
                    for r in range(N_CORES):
                        kts = []
                        for oc in range(HC):
                            kt_t = pKT.tile([P, S_LOC], BF16, name="kt", tag="kt")
                            nc.sync.dma_start(
                                kt_t[:],
                                kag[r * H + oc * P: r * H + (oc + 1) * P, cb],
                            )
                            kts.append(kt_t)
                        for j in range(4):
                            kc = r * 4 + j
                            ps_s = pp.tile([P, S_LOC], F32, name="ps_s", tag="bank")
                            for oc in range(HC):
                                nc.tensor.matmul(
                                    ps_s[:],
                                    kts[oc][:, j * P:(j + 1) * P],
                                    qT_sb[oc][:, cb],
                                    start=(oc == 0), stop=(oc == HC - 1),
                                )
                            e_t = pexp.tile([P, S_LOC], BF16, name="exp", tag="exp")
                            nc.scalar.activation(e_t[:], ps_s[:], AF.Exp, scale=SCALE)
                            exp_tiles.append(e_t)
                            nc.tensor.matmul(
                                dmat_ps[:], ones128[:], e_t[:],
                                start=(kc == 0), stop=(kc == NKC - 1),
                            )
                    recipd = prd.tile([P, S_LOC], F32, name="recipd", tag="rd")
                    nc.vector.reciprocal(recipd[:], dmat_ps[:])
                    for kc in range(NKC):
                        nc.vector.tensor_mul(
                            exp_tiles[kc][:], exp_tiles[kc][:], recipd[:]
                        )

                    # ---- attnT accumulation over all keys ----
                    attn_ps = [
                        pp.tile([P, S_LOC], F32, name=f"pa{h2}", tag="bank")
                        for h2 in range(HC)
                    ]
                    for kc in range(NKC):
                        r, j = divmod(kc, 4)
                        v_t = pV.tile([P, H], BF16, name="vst", tag="vst")
                        nc.sync.dma_start(
                            v_t[:],
                            vag[r * T + b * S_LOC + j * P:
                                r * T + b * S_LOC + (j + 1) * P, :],
                        )
                        for h2 in range(HC):
                            nc.tensor.matmul(
                                attn_ps[h2][:],
                                v_t[:, h2 * P:(h2 + 1) * P],
                                exp_tiles[kc][:],
                                start=(kc == 0), stop=(kc == NKC - 1),
                            )
                    for h2 in range(HC):
                        nc.scalar.activation(
                            attnT[h2][:, cb], attn_ps[h2][:], AF.Copy
                        )

                # ---- output projection ----
                wo = []
                for h in range(HC):
                    w_t = pwo.tile([P, H], F32R, name=f"wo{h}")
                    nc.sync.dma_start(w_t[:], woT_d[h * P:(h + 1) * P, :])
                    wo.append(w_t)
                for ts in range(HC):
                    for oc2 in range(2):
                        po = pp.tile([P, 512], F32, name="po", tag="bank")
                        for h2 in range(HC):
                            nc.tensor.matmul(
                                po[:],
                                attnT[h2][:, ts * P:(ts + 1) * P],
                                wo[h2][:, oc2 * 512:(oc2 + 1) * 512],
                                start=(h2 == 0), stop=(h2 == HC - 1),
                            )
                        o_t = pout.tile([P, 512], F32, name="ot", tag="ot")
                        nc.vector.tensor_add(
                            o_t[:], po[:], bo_bc[:, oc2 * 512:(oc2 + 1) * 512]
                        )
                        nc.sync.dma_start(
                            out_d[ts * P:(ts + 1) * P,
                                  oc2 * 512:(oc2 + 1) * 512],
                            o_t[:],
                        )

    nc.compile()
    return nc


def make_in_maps(x, Wq, bq, Wk, bk, Wv, bv, Wo, bo):
    x = np.asarray(x, np.float32)
    wqT = np.ascontiguousarray(np.asarray(Wq, np.float32).T)
    wkT = np.ascontiguousarray(np.asarray(Wk, np.float32).T)
    wvT = np.ascontiguousarray(np.asarray(Wv, np.float32).T)
    woT = np.ascontiguousarray(np.asarray(Wo, np.float32).T)
    common = dict(
        wqT=wqT, wkT=wkT, wvT=wvT, woT=woT,
        bq=np.asarray(bq, np.float32), bk=np.asarray(bk, np.float32),
        bv=np.asarray(bv, np.float32), bo=np.asarray(bo, np.float32),
    )
    in_maps = []
    for r in range(N_CORES):
        xr = x[:, r * S_LOC:(r + 1) * S_LOC, :].reshape(T, H)
        in_maps.append(dict(xT=np.ascontiguousarray(xr.T), **common))
    return in_maps


def assemble(results):
    shards = np.stack([res["out"] for res in results])      # [R, T, H]
    return np.ascontiguousarray(
        shards.reshape(N_CORES, B, S_LOC, H).transpose(1, 0, 2, 3).reshape(B, S, H)
    )


def kernel(x, Wq, bq, Wk, bk, Wv, bv, Wo, bo):
    if "nc" not in _CACHE:
        _CACHE["nc"] = build_program()
    nc = _CACHE["nc"]
    in_maps = make_in_maps(x, Wq, bq, Wk, bk, Wv, bv, Wo, bo)
    res = run_bass_kernel_spmd(nc, in_maps, core_ids=list(range(N_CORES)))
    return assemble(res.results)


# revision 15
# speedup vs baseline: 70.7523x; 70.7523x over previous
"""Distributed full-hidden attention on 8 trn2 NeuronCores.

Math (per reference):
  q = x @ Wq.T + bq ; k, v likewise          [B, S, H]
  scores = q @ k.T / sqrt(64)                [B, S, S]  (full hidden dim)
  out = softmax(scores) @ v @ Wo.T + bo      [B, S, H]

Weight folding (host side): with M = Wq.T @ Wk and N = Wv.T @ Wo.T,
  scores = x M x.T + (x Wq.T bk).1^T + 1.(x Wk.T bq)^T + const
The query-side bias term is constant along keys, so softmax drops it;
the key-side term c = x @ (Wk.T bq) survives as an exp bias. Similarly
  softmax(.) @ v @ Wo.T + bo = (1/d) (E @ x) @ N + (Wo bv + bo)
with E the unnormalized exp weights and d its row sums. So the kernel
needs NO k/v/o projections and NO collectives: each core gets the full
x (bf16, host-replicated), computes g = x_local @ M (fp32r), the two
big bf16 matmuls E = exp(x g.T/8) and U = E.T-style accumulation, then
U @ N. Sequence-parallel: core r owns queries x[:, r*512:(r+1)*512, :].

Precision: g-proj and out-proj on the PE in fp32r (full rate, ~1.5e-4
matmul rel err); the two big attention matmuls in bf16 (end-to-end rel
err ~6e-3 vs the fp32 reference).

Layouts per core (t = b*512 + s_local, 1024 local tokens; tau = global
token b*4096 + s):
  xT   [H, T]    fp32r (host-transposed local shard)   -> g-proj lhs
  xtf  [H, B*S]  bf16  (full x, h-major)               -> scores lhsT
  xf   [B*S, H]  bf16  (full x, natural)               -> attn lhsT
  gT[o, t] = M.T-as-lhsT proj of xT
  scoresT[sk, tq] = xtf.T @ gT ; exp on ACT (scale=1/8, bias=c/8)
  d[1, tq] = ones.T @ exp  (psum accumulate over all keys)
  attnUT[h, tq] += xf(block).T-as-lhsT @ exp  (per-block psum -> DVE add)
  out[t, o] = (attnUT.T @ N) * (1/d) + (Wo bv + bo)
"""

import numpy as np
import ml_dtypes

import concourse.mybir as mybir
import concourse.tile as tile
from concourse import bacc
from concourse.bass_utils import run_bass_kernel_spmd

N_CORES = 8
B, S, H = 2, 4096, 1024
S_LOC = S // N_CORES      # 512 tokens per batch per core
T = B * S_LOC             # 1024 local tokens
P = 128
HC = H // P               # 8 chunks of the hidden dim
NKC = S // P              # 32 key chunks of 128 per batch
NKG = S // 512            # 8 key groups of 512 per batch
F32 = mybir.dt.float32
F32R = mybir.dt.float32r
BF16 = mybir.dt.bfloat16
AF = mybir.ActivationFunctionType
ALU = mybir.AluOpType
SCALE = 1.0 / 8.0         # 1/sqrt(HEAD_DIM=64)

_CACHE = {}


def build_program():
    nc = bacc.Bacc(
        "TRN2",
        target_bir_lowering=False,
        debug=False,
        enable_asserts=False,
        num_devices=N_CORES,
    )

    xT_d = nc.dram_tensor("xT", [H, T], F32R, kind="ExternalInput").ap()
    m_d = nc.dram_tensor("m_mat", [H, H], F32R, kind="ExternalInput").ap()
    n_d = nc.dram_tensor("n_mat", [H, H], F32R, kind="ExternalInput").ap()
    xtf_d = nc.dram_tensor("xtf", [H, B * S], BF16, kind="ExternalInput").ap()
    xf_d = nc.dram_tensor("xf", [B * S, H], BF16, kind="ExternalInput").ap()
    c8_d = nc.dram_tensor("c8", [B * S], F32, kind="ExternalInput").ap()
    bout_d = nc.dram_tensor("b_out", [H], F32, kind="ExternalInput").ap()
    out_d = nc.dram_tensor("out", [T, H], F32, kind="ExternalOutput").ap()

    with tile.TileContext(nc) as tc:
        with (
            tc.tile_pool(name="psum", bufs=8, space="PSUM") as pp,
            tc.tile_pool(name="misc", bufs=1) as pmisc,
            tc.tile_pool(name="gTp", bufs=1) as pgT,
        ):
            # --- small constants / biases -------------------------------
            ones1 = pmisc.tile([1, P], F32, name="ones1")
            nc.vector.memset(ones1[:], 1.0)
            onescol = pmisc.tile([P, 1], BF16, name="onescol")
            nc.vector.memset(onescol[:], 1.0)

            # c/8 per key token: [128, 64] (col = global key chunk)
            c8_sb = pmisc.tile([P, B * S // P], F32, name="c8_sb")
            nc.sync.dma_start(c8_sb[:], c8_d.rearrange("(a p) -> p a", p=P))

            bout_row = pmisc.tile([1, H], F32, name="bout_row")
            nc.sync.dma_start(bout_row[:], bout_d.rearrange("(o h) -> o h", o=1))
            bout_bc = pmisc.tile([P, H], F32, name="bout_bc")
            for c2 in range(2):
                ps = pp.tile([P, 512], F32, name="ps_b", tag="bank")
                nc.tensor.matmul(
                    ps[:], ones1[:], bout_row[:, c2 * 512:(c2 + 1) * 512],
                    start=True, stop=True,
                )
                nc.scalar.activation(
                    bout_bc[:, c2 * 512:(c2 + 1) * 512], ps[:], AF.Copy
                )

            gT_sb = [pgT.tile([P, T], BF16, name=f"gT{h}") for h in range(HC)]
            dT_sb = pmisc.tile([P, B * 4], F32, name="dT_sb")

            # =========== phase A: g = x_local @ M  (gT layout) ==========
            with (
                tc.tile_pool(name="xTp", bufs=1) as pxT,
                tc.tile_pool(name="wp", bufs=8) as pw,
            ):
                xT_sb = []
                for h in range(HC):
                    t_ = pxT.tile([P, T], F32R, name=f"xT{h}")
                    nc.sync.dma_start(t_[:], xT_d[h * P:(h + 1) * P, :])
                    xT_sb.append(t_)
                m_sb = []
                for h in range(HC):
                    w_t = pw.tile([P, H], F32R, name="w", tag="w")
                    nc.sync.dma_start(w_t[:], m_d[h * P:(h + 1) * P, :])
                    m_sb.append(w_t)
                for tc2 in range(2):
                    for oc in range(HC):
                        ps = pp.tile([P, 512], F32, name="ps_p", tag="bank")
                        for h in range(HC):
                            nc.tensor.matmul(
                                ps[:],
                                m_sb[h][:, oc * P:(oc + 1) * P],
                                xT_sb[h][:, tc2 * 512:(tc2 + 1) * 512],
                                start=(h == 0), stop=(h == HC - 1),
                            )
                        nc.scalar.activation(
                            gT_sb[oc][:, tc2 * 512:(tc2 + 1) * 512],
                            ps[:], AF.Copy,
                        )

            # =========== phase B: attention =============================
            with (
                tc.tile_pool(name="ktp", bufs=16) as pKT,
                tc.tile_pool(name="expp", bufs=10) as pexp,
                tc.tile_pool(name="vstp", bufs=8) as pV,
                tc.tile_pool(name="attp", bufs=1) as pattn,
                tc.tile_pool(name="wop", bufs=1) as pwo,
                tc.tile_pool(name="outp", bufs=4) as pout,
                tc.tile_pool(name="rdp", bufs=2) as prd,
            ):
                attnT = [pattn.tile([P, T], F32R, name=f"attnT{h}")
                         for h in range(HC)]
                n_sb = []
                for h in range(HC):
                    w_t = pwo.tile([P, H], F32R, name=f"wo{h}")
                    nc.sync.dma_start(w_t[:], n_d[h * P:(h + 1) * P, :])
                    n_sb.append(w_t)

                for b in range(B):
                    cb = slice(b * S_LOC, (b + 1) * S_LOC)
                    dpsum = pp.tile([1, S_LOC], F32, name="dps", tag="bank")
                    for kg in range(NKG):
                        tau0 = b * S + kg * 512
                        kts = []
                        for oc in range(HC):
                            kt_t = pKT.tile([P, 512], BF16, name="kt",
                                            tag="kt")
                            nc.sync.dma_start(
                                kt_t[:],
                                xtf_d[oc * P:(oc + 1) * P, tau0:tau0 + 512],
                            )
                            kts.append(kt_t)
                        vts = []
                        for j in range(4):
                            v_t = pV.tile([P, H], BF16, name="vst", tag="vst")
                            nc.sync.dma_start(
                                v_t[:],
                                xf_d[tau0 + j * P: tau0 + (j + 1) * P, :],
                            )
                            vts.append(v_t)
                        es = []
                        for j in range(4):
                            kc = kg * 4 + j
                            ps_s = pp.tile([P, S_LOC], F32, name="ps_s",
                                           tag="bank")
                            for oc in range(HC):
                                nc.tensor.matmul(
                                    ps_s[:],
                                    kts[oc][:, j * P:(j + 1) * P],
                                    gT_sb[oc][:, cb],
                                    start=(oc == 0), stop=(oc == HC - 1),
                                )
                            e_t = pexp.tile([P, S_LOC], BF16, name="exp",
                                            tag="exp")
                            nc.scalar.activation(
                                e_t[:], ps_s[:], AF.Exp, scale=SCALE,
                                bias=c8_sb[:, b * 32 + kc: b * 32 + kc + 1],
                            )
                            es.append(e_t)
                            nc.tensor.matmul(
                                dpsum[:], onescol[:], e_t[:],
                                start=(kc == 0), stop=(kc == NKC - 1),
                            )
                        for h2 in range(HC):
                            pa = pp.tile([P, S_LOC], F32, name="pa",
                                         tag="bank")
                            for j in range(4):
                                nc.tensor.matmul(
                                    pa[:],
                                    vts[j][:, h2 * P:(h2 + 1) * P],
                                    es[j][:],
                                    start=(j == 0), stop=(j == 3),
                                )
                            if kg == 0:
                                nc.vector.tensor_copy(attnT[h2][:, cb], pa[:])
                            else:
                                nc.vector.tensor_add(
                                    attnT[h2][:, cb], pa[:], attnT[h2][:, cb]
                                )
                    d_sb = prd.tile([1, S_LOC], F32, name="d_sb", tag="rd")
                    nc.vector.reciprocal(d_sb[:], dpsum[:])
                    for tt in range(4):
                        nc.sync.dma_start(
                            dT_sb[:, b * 4 + tt: b * 4 + tt + 1],
                            d_sb[0:1, tt * P:(tt + 1) * P],
                        )

                    # ---- output projection for this batch's tokens ----
                    for ts in range(b * 4, b * 4 + 4):
                        for oc2 in range(2):
                            po = pp.tile([P, 512], F32, name="po", tag="bank")
                            for h2 in range(HC):
                                nc.tensor.matmul(
                                    po[:],
                                    attnT[h2][:, ts * P:(ts + 1) * P],
                                    n_sb[h2][:, oc2 * 512:(oc2 + 1) * 512],
                                    start=(h2 == 0), stop=(h2 == HC - 1),
                                )
                            o_t = pout.tile([P, 512], F32, name="ot", tag="ot")
                            nc.vector.scalar_tensor_tensor(
                                o_t[:], po[:], dT_sb[:, ts:ts + 1],
                                bout_bc[:, oc2 * 512:(oc2 + 1) * 512],
                                ALU.mult, ALU.add,
                            )
                            nc.sync.dma_start(
                                out_d[ts * P:(ts + 1) * P,
                                      oc2 * 512:(oc2 + 1) * 512],
                                o_t[:],
                            )

    nc.compile()
    return nc


def make_in_maps(x, Wq, bq, Wk, bk, Wv, bv, Wo, bo):
    x = np.asarray(x, np.float32)
    Wq = np.asarray(Wq, np.float32)
    Wk = np.asarray(Wk, np.float32)
    Wv = np.asarray(Wv, np.float32)
    Wo = np.asarray(Wo, np.float32)
    bq = np.asarray(bq, np.float32)
    bk = np.asarray(bk, np.float32)
    bv = np.asarray(bv, np.float32)
    bo = np.asarray(bo, np.float32)

    m_mat = np.ascontiguousarray((Wq.T @ Wk))                 # [h_in, h_in2]
    n_mat = np.ascontiguousarray(Wv.T @ Wo.T)                 # [h_in, o]
    xfull = x.reshape(B * S, H)                               # tau-major
    xf = xfull.astype(ml_dtypes.bfloat16)
    xtf = np.ascontiguousarray(xfull.T).astype(ml_dtypes.bfloat16)
    c8 = (xfull @ (Wk.T @ bq)) * np.float32(SCALE)            # key-side bias
    b_out = Wo @ bv + bo

    common = dict(m_mat=m_mat, n_mat=n_mat, xtf=xtf, xf=xf,
                  c8=c8.astype(np.float32), b_out=b_out.astype(np.float32))
    in_maps = []
    for r in range(N_CORES):
        xr = x[:, r * S_LOC:(r + 1) * S_LOC, :].reshape(T, H)
        in_maps.append(dict(xT=np.ascontiguousarray(xr.T), **common))
    return in_maps


def assemble(results):
    shards = np.stack([res["out"] for res in results])      # [R, T, H]
    return np.ascontiguousarray(
        shards.reshape(N_CORES, B, S_LOC, H).transpose(1, 0, 2, 3)
        .reshape(B, S, H)
    )


def kernel(x, Wq, bq, Wk, bk, Wv, bv, Wo, bo):
    if "nc" not in _CACHE:
        _CACHE["nc"] = build_program()
    nc = _CACHE["nc"]
    in_maps = make_in_maps(x, Wq, bq, Wk, bk, Wv, bv, Wo, bo)
    res = run_bass_kernel_spmd(nc, in_maps, core_ids=list(range(N_CORES)))
    return assemble(res.results)


# revision 16
# speedup vs baseline: 70.7856x; 1.0005x over previous
"""Distributed full-hidden attention on 8 trn2 NeuronCores.

Math (per reference):
  q = x @ Wq.T + bq ; k, v likewise          [B, S, H]
  scores = q @ k.T / sqrt(64)                [B, S, S]  (full hidden dim)
  out = softmax(scores) @ v @ Wo.T + bo      [B, S, H]

Weight folding (host side): with M = Wq.T @ Wk and N = Wv.T @ Wo.T,
  scores = x M x.T + (x Wq.T bk).1^T + 1.(x Wk.T bq)^T + const
The query-side bias term is constant along keys, so softmax drops it;
the key-side term c = x @ (Wk.T bq) survives as an exp bias. Similarly
  softmax(.) @ v @ Wo.T + bo = (1/d) (E @ x) @ N + (Wo bv + bo)
with E the unnormalized exp weights and d its row sums. So the kernel
needs NO k/v/o projections and NO collectives: each core gets the full
x (bf16, host-replicated), computes g = x_local @ M (fp32r), the two
big bf16 matmuls E = exp(x g.T/8) and U = E.T-style accumulation, then
U @ N. Sequence-parallel: core r owns queries x[:, r*512:(r+1)*512, :].

Precision: g-proj and out-proj on the PE in fp32r (full rate, ~1.5e-4
matmul rel err); the two big attention matmuls in bf16 (end-to-end rel
err ~6e-3 vs the fp32 reference).

Layouts per core (t = b*512 + s_local, 1024 local tokens; tau = global
token b*4096 + s):
  xT   [H, T]    fp32r (host-transposed local shard)   -> g-proj lhs
  xtf  [H, B*S]  bf16  (full x, h-major)               -> scores lhsT
  xf   [B*S, H]  bf16  (full x, natural)               -> attn lhsT
  gT[o, t] = M.T-as-lhsT proj of xT
  scoresT[sk, tq] = xtf.T @ gT ; exp on ACT (scale=1/8, bias=c/8)
  d[1, tq] = ones.T @ exp  (psum accumulate over all keys)
  attnUT[h, tq] += xf(block).T-as-lhsT @ exp  (per-block psum -> DVE add)
  out[t, o] = (attnUT.T @ N) * (1/d) + (Wo bv + bo)
"""

import numpy as np
import ml_dtypes

import concourse.mybir as mybir
import concourse.tile as tile
from concourse import bacc
from concourse.bass_utils import run_bass_kernel_spmd

N_CORES = 8
B, S, H = 2, 4096, 1024
S_LOC = S // N_CORES      # 512 tokens per batch per core
T = B * S_LOC             # 1024 local tokens
P = 128
HC = H // P               # 8 chunks of the hidden dim
NKC = S // P              # 32 key chunks of 128 per batch
NKG = S // 512            # 8 key groups of 512 per batch
F32 = mybir.dt.float32
F32R = mybir.dt.float32r
BF16 = mybir.dt.bfloat16
AF = mybir.ActivationFunctionType
ALU = mybir.AluOpType
SCALE = 1.0 / 8.0         # 1/sqrt(HEAD_DIM=64)

_CACHE = {}


def build_program():
    nc = bacc.Bacc(
        "TRN2",
        target_bir_lowering=False,
        debug=False,
        enable_asserts=False,
        num_devices=N_CORES,
    )

    xT_d = nc.dram_tensor("xT", [H, T], F32R, kind="ExternalInput").ap()
    m_d = nc.dram_tensor("m_mat", [H, H], F32R, kind="ExternalInput").ap()
    n_d = nc.dram_tensor("n_mat", [H, H], F32R, kind="ExternalInput").ap()
    xtf_d = nc.dram_tensor("xtf", [H, B * S], BF16, kind="ExternalInput").ap()
    xf_d = nc.dram_tensor("xf", [B * S, H], BF16, kind="ExternalInput").ap()
    c8_d = nc.dram_tensor("c8", [B * S], F32, kind="ExternalInput").ap()
    bout_d = nc.dram_tensor("b_out", [H], F32, kind="ExternalInput").ap()
    out_d = nc.dram_tensor("out", [T, H], F32, kind="ExternalOutput").ap()

    with tile.TileContext(nc) as tc:
        with (
            tc.tile_pool(name="psum", bufs=8, space="PSUM") as pp,
            tc.tile_pool(name="misc", bufs=1) as pmisc,
            tc.tile_pool(name="gTp", bufs=1) as pgT,
        ):
            # --- small constants / biases -------------------------------
            ones1 = pmisc.tile([1, P], F32, name="ones1")
            nc.vector.memset(ones1[:], 1.0)
            onescol = pmisc.tile([P, 1], BF16, name="onescol")
            nc.vector.memset(onescol[:], 1.0)

            # c/8 per key token: [128, 64] (col = global key chunk)
            c8_sb = pmisc.tile([P, B * S // P], F32, name="c8_sb")
            nc.sync.dma_start(c8_sb[:], c8_d.rearrange("(a p) -> p a", p=P))

            bout_row = pmisc.tile([1, H], F32, name="bout_row")
            nc.sync.dma_start(bout_row[:], bout_d.rearrange("(o h) -> o h", o=1))
            bout_bc = pmisc.tile([P, H], F32, name="bout_bc")
            for c2 in range(2):
                ps = pp.tile([P, 512], F32, name="ps_b", tag="bank")
                nc.tensor.matmul(
                    ps[:], ones1[:], bout_row[:, c2 * 512:(c2 + 1) * 512],
                    start=True, stop=True,
                )
                nc.scalar.activation(
                    bout_bc[:, c2 * 512:(c2 + 1) * 512], ps[:], AF.Copy
                )

            gT_sb = [pgT.tile([P, T], BF16, name=f"gT{h}") for h in range(HC)]
            dT_sb = pmisc.tile([P, B * 4], F32, name="dT_sb")

            # =========== phase A: g = x_local @ M  (gT layout) ==========
            with (
                tc.tile_pool(name="xTp", bufs=1) as pxT,
                tc.tile_pool(name="wp", bufs=8) as pw,
            ):
                xT_sb = []
                for h in range(HC):
                    t_ = pxT.tile([P, T], F32R, name=f"xT{h}")
                    nc.sync.dma_start(t_[:], xT_d[h * P:(h + 1) * P, :])
                    xT_sb.append(t_)
                m_sb = []
                for h in range(HC):
                    w_t = pw.tile([P, H], F32R, name="w", tag="w")
                    nc.sync.dma_start(w_t[:], m_d[h * P:(h + 1) * P, :])
                    m_sb.append(w_t)
                for tc2 in range(2):
                    for oc in range(HC):
                        ps = pp.tile([P, 512], F32, name="ps_p", tag="bank")
                        for h in range(HC):
                            nc.tensor.matmul(
                                ps[:],
                                m_sb[h][:, oc * P:(oc + 1) * P],
                                xT_sb[h][:, tc2 * 512:(tc2 + 1) * 512],
                                start=(h == 0), stop=(h == HC - 1),
                            )
                        nc.scalar.activation(
                            gT_sb[oc][:, tc2 * 512:(tc2 + 1) * 512],
                            ps[:], AF.Copy,
                        )

            # =========== phase B: attention =============================
            with (
                tc.tile_pool(name="ktp", bufs=24) as pKT,
                tc.tile_pool(name="expp", bufs=12) as pexp,
                tc.tile_pool(name="vstp", bufs=12) as pV,
                tc.tile_pool(name="attp", bufs=1) as pattn,
                tc.tile_pool(name="wop", bufs=1) as pwo,
                tc.tile_pool(name="outp", bufs=4) as pout,
                tc.tile_pool(name="rdp", bufs=2) as prd,
            ):
                attnT = [pattn.tile([P, T], F32R, name=f"attnT{h}")
                         for h in range(HC)]
                n_sb = []
                for h in range(HC):
                    w_t = pwo.tile([P, H], F32R, name=f"wo{h}")
                    nc.sync.dma_start(w_t[:], n_d[h * P:(h + 1) * P, :])
                    n_sb.append(w_t)

                for b in range(B):
                    cb = slice(b * S_LOC, (b + 1) * S_LOC)
                    dpsum = pp.tile([1, S_LOC], F32, name="dps", tag="bank")
                    for kg in range(NKG):
                        tau0 = b * S + kg * 512
                        kts = []
                        for oc in range(HC):
                            kt_t = pKT.tile([P, 512], BF16, name="kt",
                                            tag="kt")
                            nc.sync.dma_start(
                                kt_t[:],
                                xtf_d[oc * P:(oc + 1) * P, tau0:tau0 + 512],
                            )
                            kts.append(kt_t)
                        vts = []
                        for j in range(4):
                            v_t = pV.tile([P, H], BF16, name="vst", tag="vst")
                            nc.sync.dma_start(
                                v_t[:],
                                xf_d[tau0 + j * P: tau0 + (j + 1) * P, :],
                            )
                            vts.append(v_t)
                        es = []
                        for j in range(4):
                            kc = kg * 4 + j
                            ps_s = pp.tile([P, S_LOC], F32, name="ps_s",
                                           tag="bank")
                            for oc in range(HC):
                                nc.tensor.matmul(
                                    ps_s[:],
                                    kts[oc][:, j * P:(j + 1) * P],
                                    gT_sb[oc][:, cb],
                                    start=(oc == 0), stop=(oc == HC - 1),
                                )
                            e_t = pexp.tile([P, S_LOC], BF16, name="exp",
                                            tag="exp")
                            nc.scalar.activation(
                                e_t[:], ps_s[:], AF.Exp, scale=SCALE,
                                bias=c8_sb[:, b * 32 + kc: b * 32 + kc + 1],
                            )
                            es.append(e_t)
                            nc.tensor.matmul(
                                dpsum[:], onescol[:], e_t[:],
                                start=(kc == 0), stop=(kc == NKC - 1),
                            )
                        for h2 in range(HC):
                            pa = pp.tile([P, S_LOC], F32, name="pa",
                                         tag="bank")
                            for j in range(4):
                                nc.tensor.matmul(
                                    pa[:],
                                    vts[j][:, h2 * P:(h2 + 1) * P],
                                    es[j][:],
                                    start=(j == 0), stop=(j == 3),
                                )
                            if kg == 0:
                                nc.vector.tensor_copy(attnT[h2][:, cb], pa[:])
                            else:
                                nc.vector.tensor_add(
                                    attnT[h2][:, cb], pa[:], attnT[h2][:, cb]
                                )
                    d_sb = prd.tile([1, S_LOC], F32, name="d_sb", tag="rd")
                    nc.vector.reciprocal(d_sb[:], dpsum[:])
                    for tt in range(4):
                        nc.sync.dma_start(
                            dT_sb[:, b * 4 + tt: b * 4 + tt + 1],
                            d_sb[0:1, tt * P:(tt + 1) * P],
                        )

                    # ---- output projection for this batch's tokens ----
                    for ts in range(b * 4, b * 4 + 4):
                        for oc2 in range(2):
                            po = pp.tile([P, 512], F32, name="po", tag="bank")
                            for h2 in range(HC):
                                nc.tensor.matmul(
                                    po[:],
                                    attnT[h2][:, ts * P:(ts + 1) * P],
                                    n_sb[h2][:, oc2 * 512:(oc2 + 1) * 512],
                                    start=(h2 == 0), stop=(h2 == HC - 1),
                                )
                            o_t = pout.tile([P, 512], F32, name="ot", tag="ot")
                            nc.vector.scalar_tensor_tensor(
                                o_t[:], po[:], dT_sb[:, ts:ts + 1],
                                bout_bc[:, oc2 * 512:(oc2 + 1) * 512],
                                ALU.mult, ALU.add,
                            )
                            nc.sync.dma_start(
                                out_d[ts * P:(ts + 1) * P,
                                      oc2 * 512:(oc2 + 1) * 512],
                                o_t[:],
                            )

    nc.compile()
    return nc


def make_in_maps(x, Wq, bq, Wk, bk, Wv, bv, Wo, bo):
    x = np.asarray(x, np.float32)
    Wq = np.asarray(Wq, np.float32)
    Wk = np.asarray(Wk, np.float32)
    Wv = np.asarray(Wv, np.float32)
    Wo = np.asarray(Wo, np.float32)
    bq = np.asarray(bq, np.float32)
    bk = np.asarray(bk, np.float32)
    bv = np.asarray(bv, np.float32)
    bo = np.asarray(bo, np.float32)

    m_mat = np.ascontiguousarray((Wq.T @ Wk))                 # [h_in, h_in2]
    n_mat = np.ascontiguousarray(Wv.T @ Wo.T)                 # [h_in, o]
    xfull = x.reshape(B * S, H)                               # tau-major
    xf = xfull.astype(ml_dtypes.bfloat16)
    xtf = np.ascontiguousarray(xfull.T).astype(ml_dtypes.bfloat16)
    c8 = (xfull @ (Wk.T @ bq)) * np.float32(SCALE)            # key-side bias
    b_out = Wo @ bv + bo

    common = dict(m_mat=m_mat, n_mat=n_mat, xtf=xtf, xf=xf,
                  c8=c8.astype(np.float32), b_out=b_out.astype(np.float32))
    in_maps = []
    for r in range(N_CORES):
        xr = x[:, r * S_LOC:(r + 1) * S_LOC, :].reshape(T, H)
        in_maps.append(dict(xT=np.ascontiguousarray(xr.T), **common))
    return in_maps


def assemble(results):
    shards = np.stack([res["out"] for res in results])      # [R, T, H]
    return np.ascontiguousarray(
        shards.reshape(N_CORES, B, S_LOC, H).transpose(1, 0, 2, 3)
        .reshape(B, S, H)
    )


def kernel(x, Wq, bq, Wk, bk, Wv, bv, Wo, bo):
    if "nc" not in _CACHE:
        _CACHE["nc"] = build_program()
    nc = _CACHE["nc"]
    in_maps = make_in_maps(x, Wq, bq, Wk, bk, Wv, bv, Wo, bo)
    res = run_bass_kernel_spmd(nc, in_maps, core_ids=list(range(N_CORES)))
    return assemble(res.results)


# revision 19
# speedup vs baseline: 73.5087x; 1.0385x over previous
"""Distributed full-hidden attention on 8 trn2 NeuronCores.

Math (per reference):
  q = x @ Wq.T + bq ; k, v likewise          [B, S, H]
  scores = q @ k.T / sqrt(64)                [B, S, S]  (full hidden dim)
  out = softmax(scores) @ v @ Wo.T + bo      [B, S, H]

Weight folding (host side): with M = Wq.T @ Wk and N = Wv.T @ Wo.T,
  scores = x M x.T + (x Wq.T bk).1^T + 1.(x Wk.T bq)^T + const
The query-side bias term is constant along keys, so softmax drops it;
the key-side term c = x @ (Wk.T bq) survives as an exp bias. Similarly
  softmax(.) @ v @ Wo.T + bo = (1/d) (E @ x) @ N + (Wo bv + bo)
with E the unnormalized exp weights and d its row sums. So the kernel
needs NO k/v/o projections and NO collectives: each core gets the full
x (bf16, host-replicated), computes g = x_local @ M (fp32r), the two
big bf16 matmuls E = exp(x g.T/8) and U = E.T-style accumulation, then
U @ N. Sequence-parallel: core r owns queries x[:, r*512:(r+1)*512, :].

Precision: g-proj and out-proj on the PE in fp32r (full rate, ~1.5e-4
matmul rel err); the two big attention matmuls in bf16 (end-to-end rel
err ~6e-3 vs the fp32 reference).

Layouts per core (t = b*512 + s_local, 1024 local tokens; tau = global
token b*4096 + s):
  xT   [H, T]    fp32r (host-transposed local shard)   -> g-proj lhs
  xtf  [H, B*S]  bf16  (full x, h-major)               -> scores lhsT
  xf   [B*S, H]  bf16  (full x, natural)               -> attn lhsT
  gT[o, t] = M.T-as-lhsT proj of xT
  scoresT[sk, tq] = xtf.T @ gT ; exp on ACT (scale=1/8, bias=c/8)
  d[1, tq] = ones.T @ exp  (psum accumulate over all keys)
  attnUT[h, tq] += xf(block).T-as-lhsT @ exp  (per-block psum -> DVE add)
  out[t, o] = (attnUT.T @ N) * (1/d) + (Wo bv + bo)
"""

import numpy as np
import ml_dtypes

import concourse.mybir as mybir
import concourse.tile as tile
from concourse import bacc
from concourse.bass_utils import run_bass_kernel_spmd

N_CORES = 8
B, S, H = 2, 4096, 1024
S_LOC = S // N_CORES      # 512 tokens per batch per core
T = B * S_LOC             # 1024 local tokens
P = 128
HC = H // P               # 8 chunks of the hidden dim
NKC = S // P              # 32 key chunks of 128 per batch
NKG = S // 512            # 8 key groups of 512 per batch
F32 = mybir.dt.float32
F32R = mybir.dt.float32r
BF16 = mybir.dt.bfloat16
AF = mybir.ActivationFunctionType
ALU = mybir.AluOpType
SCALE = 1.0 / 8.0         # 1/sqrt(HEAD_DIM=64)

_CACHE = {}


def build_program():
    nc = bacc.Bacc(
        "TRN2",
        target_bir_lowering=False,
        debug=False,
        enable_asserts=False,
        num_devices=N_CORES,
    )

    xT_d = nc.dram_tensor("xT", [H, T], F32R, kind="ExternalInput").ap()
    m_d = nc.dram_tensor("m_mat", [H, H], F32R, kind="ExternalInput").ap()
    n_d = nc.dram_tensor("n_mat", [H, H], F32R, kind="ExternalInput").ap()
    xtf_d = nc.dram_tensor("xtf", [H, B * S], BF16, kind="ExternalInput").ap()
    xf_d = nc.dram_tensor("xf", [B * S, H], BF16, kind="ExternalInput").ap()
    c8_d = nc.dram_tensor("c8", [B * S], F32, kind="ExternalInput").ap()
    bout_d = nc.dram_tensor("b_out", [H], F32, kind="ExternalInput").ap()
    out_d = nc.dram_tensor("out", [T, H], F32, kind="ExternalOutput").ap()

    with tile.TileContext(nc) as tc:
        with (
            tc.tile_pool(name="psum", bufs=8, space="PSUM") as pp,
            tc.tile_pool(name="misc", bufs=1) as pmisc,
            tc.tile_pool(name="gTp", bufs=1) as pgT,
        ):
            # --- small constants / biases -------------------------------
            ones1 = pmisc.tile([1, P], F32, name="ones1")
            nc.vector.memset(ones1[:], 1.0)
            onescol = pmisc.tile([P, 1], BF16, name="onescol")
            nc.vector.memset(onescol[:], 1.0)

            # c/8 per key token: [128, 64] (col = global key chunk)
            c8_sb = pmisc.tile([P, B * S // P], F32, name="c8_sb")
            nc.sync.dma_start(c8_sb[:], c8_d.rearrange("(a p) -> p a", p=P))

            bout_row = pmisc.tile([1, H], F32, name="bout_row")
            nc.sync.dma_start(bout_row[:], bout_d.rearrange("(o h) -> o h", o=1))
            bout_bc = pmisc.tile([P, H], F32, name="bout_bc")
            for c2 in range(2):
                ps = pp.tile([P, 512], F32, name="ps_b", tag="bank")
                nc.tensor.matmul(
                    ps[:], ones1[:], bout_row[:, c2 * 512:(c2 + 1) * 512],
                    start=True, stop=True,
                )
                nc.scalar.activation(
                    bout_bc[:, c2 * 512:(c2 + 1) * 512], ps[:], AF.Copy
                )

            gT_sb = [pgT.tile([P, T], BF16, name=f"gT{h}") for h in range(HC)]
            dT_sb = pmisc.tile([P, B * 4], F32, name="dT_sb")

            # =========== phase A: g = x_local @ M  (gT layout) ==========
            with (
                tc.tile_pool(name="xTp", bufs=1) as pxT,
                tc.tile_pool(name="wp", bufs=8) as pw,
            ):
                # issue the first-half columns of M and xT before the second
                # halves so the first psum group's operands land in ~4MB
                xT_sb = [pxT.tile([P, T], F32R, name=f"xT{h}")
                         for h in range(HC)]
                m_sb = [pw.tile([P, H], F32R, name="w", tag="w")
                        for _ in range(HC)]
                for half in range(2):
                    cs = slice(half * 512, (half + 1) * 512)
                    for h in range(HC):
                        nc.sync.dma_start(
                            m_sb[h][:, cs], m_d[h * P:(h + 1) * P, cs]
                        )
                    for h in range(HC):
                        nc.sync.dma_start(
                            xT_sb[h][:, cs], xT_d[h * P:(h + 1) * P, cs]
                        )
                for tc2 in range(2):
                    for oc in range(HC):
                        ps = pp.tile([P, 512], F32, name="ps_p", tag="bank")
                        for h in range(HC):
                            nc.tensor.matmul(
                                ps[:],
                                m_sb[h][:, oc * P:(oc + 1) * P],
                                xT_sb[h][:, tc2 * 512:(tc2 + 1) * 512],
                                start=(h == 0), stop=(h == HC - 1),
                            )
                        nc.scalar.activation(
                            gT_sb[oc][:, tc2 * 512:(tc2 + 1) * 512],
                            ps[:], AF.Copy,
                        )

            # =========== phase B: attention =============================
            with (
                tc.tile_pool(name="ktp", bufs=24) as pKT,
                tc.tile_pool(name="expp", bufs=33) as pexp,
                tc.tile_pool(name="vstp", bufs=12) as pV,
                tc.tile_pool(name="attp", bufs=1) as pattn,
                tc.tile_pool(name="wop", bufs=1) as pwo,
                tc.tile_pool(name="outp", bufs=4) as pout,
                tc.tile_pool(name="rdp", bufs=2) as prd,
            ):
                attnT = [pattn.tile([P, T], F32R, name=f"attnT{h}")
                         for h in range(HC)]
                n_sb = []
                for h in range(HC):
                    w_t = pwo.tile([P, H], F32R, name=f"wo{h}")
                    nc.sync.dma_start(w_t[:], n_d[h * P:(h + 1) * P, :])
                    n_sb.append(w_t)

                for b in range(B):
                    cb = slice(b * S_LOC, (b + 1) * S_LOC)
                    dpsum = pp.tile([1, S_LOC], F32, name="dps", tag="bank")
                    # pass A: scores + exp + d + attn for h 0..511, with the
                    # attn halves accumulated directly in PSUM over all keys
                    paA = [pp.tile([P, S_LOC], F32, name=f"paA{h2}",
                                   tag="bank") for h2 in range(4)]
                    es_all = []
                    for kg in range(NKG):
                        tau0 = b * S + kg * 512
                        kts = []
                        for oc in range(HC):
                            kt_t = pKT.tile([P, 512], BF16, name="kt",
                                            tag="kt")
                            nc.sync.dma_start(
                                kt_t[:],
                                xtf_d[oc * P:(oc + 1) * P, tau0:tau0 + 512],
                            )
                            kts.append(kt_t)
                        vts = []
                        for j in range(4):
                            v_t = pV.tile([P, 512], BF16, name="vst",
                                          tag="vst")
                            nc.sync.dma_start(
                                v_t[:],
                                xf_d[tau0 + j * P: tau0 + (j + 1) * P, 0:512],
                            )
                            vts.append(v_t)
                        for j in range(4):
                            kc = kg * 4 + j
                            ps_s = pp.tile([P, S_LOC], F32, name="ps_s",
                                           tag="bank")
                            for oc in range(HC):
                                nc.tensor.matmul(
                                    ps_s[:],
                                    kts[oc][:, j * P:(j + 1) * P],
                                    gT_sb[oc][:, cb],
                                    start=(oc == 0), stop=(oc == HC - 1),
                                )
                            e_t = pexp.tile([P, S_LOC], BF16, name="exp",
                                            tag="exp")
                            nc.scalar.activation(
                                e_t[:], ps_s[:], AF.Exp, scale=SCALE,
                                bias=c8_sb[:, b * 32 + kc: b * 32 + kc + 1],
                            )
                            es_all.append(e_t)
                            nc.tensor.matmul(
                                dpsum[:], onescol[:], e_t[:],
                                start=(kc == 0), stop=(kc == NKC - 1),
                            )
                            for h2 in range(4):
                                nc.tensor.matmul(
                                    paA[h2][:],
                                    vts[j][:, h2 * P:(h2 + 1) * P],
                                    e_t[:],
                                    start=(kc == 0), stop=(kc == NKC - 1),
                                )
                    for h2 in range(4):
                        nc.scalar.activation(attnT[h2][:, cb], paA[h2][:],
                                             AF.Copy)

                    # pass B: attn for h 512..1023, re-reading the exp tiles
                    paB = [pp.tile([P, S_LOC], F32, name=f"paB{h2}",
                                   tag="bank") for h2 in range(4)]
                    for kg in range(NKG):
                        tau0 = b * S + kg * 512
                        for j in range(4):
                            kc = kg * 4 + j
                            v_t = pV.tile([P, 512], BF16, name="vst",
                                          tag="vst")
                            nc.sync.dma_start(
                                v_t[:],
                                xf_d[tau0 + j * P: tau0 + (j + 1) * P,
                                     512:1024],
                            )
                            for h2 in range(4):
                                nc.tensor.matmul(
                                    paB[h2][:],
                                    v_t[:, h2 * P:(h2 + 1) * P],
                                    es_all[kc][:],
                                    start=(kc == 0), stop=(kc == NKC - 1),
                                )
                    for h2 in range(4):
                        nc.scalar.activation(attnT[4 + h2][:, cb], paB[h2][:],
                                             AF.Copy)
                    d_sb = prd.tile([1, S_LOC], F32, name="d_sb", tag="rd")
                    nc.vector.reciprocal(d_sb[:], dpsum[:])
                    for tt in range(4):
                        nc.sync.dma_start(
                            dT_sb[:, b * 4 + tt: b * 4 + tt + 1],
                            d_sb[0:1, tt * P:(tt + 1) * P],
                        )

                    # ---- output projection for this batch's tokens ----
                    for ts in range(b * 4, b * 4 + 4):
                        for oc2 in range(2):
                            po = pp.tile([P, 512], F32, name="po", tag="bank")
                            for h2 in range(HC):
                                nc.tensor.matmul(
                                    po[:],
                                    attnT[h2][:, ts * P:(ts + 1) * P],
                                    n_sb[h2][:, oc2 * 512:(oc2 + 1) * 512],
                                    start=(h2 == 0), stop=(h2 == HC - 1),
                                )
                            o_t = pout.tile([P, 512], F32, name="ot", tag="ot")
                            nc.vector.scalar_tensor_tensor(
                                o_t[:], po[:], dT_sb[:, ts:ts + 1],
                                bout_bc[:, oc2 * 512:(oc2 + 1) * 512],
                                ALU.mult, ALU.add,
                            )
                            nc.sync.dma_start(
                                out_d[ts * P:(ts + 1) * P,
                                      oc2 * 512:(oc2 + 1) * 512],
                                o_t[:],
                            )

    nc.compile()
    return nc


def make_in_maps(x, Wq, bq, Wk, bk, Wv, bv, Wo, bo):
    x = np.asarray(x, np.float32)
    Wq = np.asarray(Wq, np.float32)
    Wk = np.asarray(Wk, np.float32)
    Wv = np.asarray(Wv, np.float32)
    Wo = np.asarray(Wo, np.float32)
    bq = np.asarray(bq, np.float32)
    bk = np.asarray(bk, np.float32)
    bv = np.asarray(bv, np.float32)
    bo = np.asarray(bo, np.float32)

    m_mat = np.ascontiguousarray((Wq.T @ Wk))                 # [h_in, h_in2]
    n_mat = np.ascontiguousarray(Wv.T @ Wo.T)                 # [h_in, o]
    xfull = x.reshape(B * S, H)                               # tau-major
    xf = xfull.astype(ml_dtypes.bfloat16)
    xtf = np.ascontiguousarray(xfull.T).astype(ml_dtypes.bfloat16)
    c8 = (xfull @ (Wk.T @ bq)) * np.float32(SCALE)            # key-side bias
    b_out = Wo @ bv + bo

    common = dict(m_mat=m_mat, n_mat=n_mat, xtf=xtf, xf=xf,
                  c8=c8.astype(np.float32), b_out=b_out.astype(np.float32))
    in_maps = []
    for r in range(N_CORES):
        xr = x[:, r * S_LOC:(r + 1) * S_LOC, :].reshape(T, H)
        in_maps.append(dict(xT=np.ascontiguousarray(xr.T), **common))
    return in_maps


def assemble(results):
    shards = np.stack([res["out"] for res in results])      # [R, T, H]
    return np.ascontiguousarray(
        shards.reshape(N_CORES, B, S_LOC, H).transpose(1, 0, 2, 3)
        .reshape(B, S, H)
    )


def kernel(x, Wq, bq, Wk, bk, Wv, bv, Wo, bo):
    if "nc" not in _CACHE:
        _CACHE["nc"] = build_program()
    nc = _CACHE["nc"]
    in_maps = make_in_maps(x, Wq, bq, Wk, bk, Wv, bv, Wo, bo)
    res = run_bass_kernel_spmd(nc, in_maps, core_ids=list(range(N_CORES)))
    return assemble(res.results)


# revision 21
# speedup vs baseline: 76.7785x; 1.0445x over previous
"""Distributed full-hidden attention on 8 trn2 NeuronCores.

Math (per reference):
  q = x @ Wq.T + bq ; k, v likewise          [B, S, H]
  scores = q @ k.T / sqrt(64)                [B, S, S]  (full hidden dim)
  out = softmax(scores) @ v @ Wo.T + bo      [B, S, H]

Weight folding (host side): with M = Wq.T @ Wk and N = Wv.T @ Wo.T,
  scores = x M x.T + (x Wq.T bk).1^T + 1.(x Wk.T bq)^T + const
The query-side bias term is constant along keys, so softmax drops it;
the key-side term c = x @ (Wk.T bq) survives as an exp bias. Similarly
  softmax(.) @ v @ Wo.T + bo = (1/d) (E @ x) @ N + (Wo bv + bo)
with E the unnormalized exp weights and d its row sums. So the kernel
needs NO k/v/o projections and NO collectives: each core gets the full
x (bf16, host-replicated), computes g = x_local @ M (fp32r), the two
big bf16 matmuls E = exp(x g.T/8) and U = E.T-style accumulation, then
U @ N. Sequence-parallel: core r owns queries x[:, r*512:(r+1)*512, :].

Precision: g-proj and out-proj on the PE in fp32r (full rate, ~1.5e-4
matmul rel err); the two big attention matmuls in bf16 (end-to-end rel
err ~6e-3 vs the fp32 reference).

Layouts per core (t = b*512 + s_local, 1024 local tokens; tau = global
token b*4096 + s):
  xT   [H, T]    fp32r (host-transposed local shard)   -> g-proj lhs
  xtf  [H, B*S]  bf16  (full x, h-major)               -> scores lhsT
  xf   [B*S, H]  bf16  (full x, natural)               -> attn lhsT
  gT[o, t] = M.T-as-lhsT proj of xT
  scoresT[sk, tq] = xtf.T @ gT ; exp on ACT (scale=1/8, bias=c/8)
  d[1, tq] = ones.T @ exp  (psum accumulate over all keys)
  attnUT[h, tq] += xf(block).T-as-lhsT @ exp  (per-block psum -> DVE add)
  out[t, o] = (attnUT.T @ N) * (1/d) + (Wo bv + bo)
"""

import numpy as np
import ml_dtypes

import concourse.mybir as mybir
import concourse.tile as tile
from concourse import bacc
from concourse.bass_utils import run_bass_kernel_spmd

N_CORES = 8
B, S, H = 2, 4096, 1024
S_LOC = S // N_CORES      # 512 tokens per batch per core
T = B * S_LOC             # 1024 local tokens
P = 128
HC = H // P               # 8 chunks of the hidden dim
NKC = S // P              # 32 key chunks of 128 per batch
NKG = S // 512            # 8 key groups of 512 per batch
F32 = mybir.dt.float32
F32R = mybir.dt.float32r
BF16 = mybir.dt.bfloat16
AF = mybir.ActivationFunctionType
ALU = mybir.AluOpType
SCALE = 1.0 / 8.0         # 1/sqrt(HEAD_DIM=64)

_CACHE = {}


def build_program():
    nc = bacc.Bacc(
        "TRN2",
        target_bir_lowering=False,
        debug=False,
        enable_asserts=False,
        num_devices=N_CORES,
    )

    xT_d = nc.dram_tensor("xT", [H, T], F32R, kind="ExternalInput").ap()
    m_d = nc.dram_tensor("m_mat", [H, H], F32R, kind="ExternalInput").ap()
    n_d = nc.dram_tensor("n_mat", [H, H], F32R, kind="ExternalInput").ap()
    xtf_d = nc.dram_tensor("xtf", [H, B * S], BF16, kind="ExternalInput").ap()
    xf_d = nc.dram_tensor("xf", [B * S, H], BF16, kind="ExternalInput").ap()
    c8_d = nc.dram_tensor("c8", [B * S], F32, kind="ExternalInput").ap()
    bout_d = nc.dram_tensor("b_out", [H], F32, kind="ExternalInput").ap()
    out_d = nc.dram_tensor("out", [T, H], F32, kind="ExternalOutput").ap()

    with tile.TileContext(nc) as tc:
        with (
            tc.tile_pool(name="psum", bufs=8, space="PSUM") as pp,
            tc.tile_pool(name="misc", bufs=1) as pmisc,
            tc.tile_pool(name="gTp", bufs=1) as pgT,
        ):
            # --- small constants / biases -------------------------------
            ones1 = pmisc.tile([1, P], F32, name="ones1")
            nc.vector.memset(ones1[:], 1.0)
            onescol = pmisc.tile([P, 1], BF16, name="onescol")
            nc.vector.memset(onescol[:], 1.0)

            # c/8 per key token: [128, 64] (col = global key chunk)
            c8_sb = pmisc.tile([P, B * S // P], F32, name="c8_sb")
            nc.sync.dma_start(c8_sb[:], c8_d.rearrange("(a p) -> p a", p=P))

            bout_row = pmisc.tile([1, H], F32, name="bout_row")
            nc.sync.dma_start(bout_row[:], bout_d.rearrange("(o h) -> o h", o=1))
            bout_bc = pmisc.tile([P, H], F32, name="bout_bc")
            for c2 in range(2):
                ps = pp.tile([P, 512], F32, name="ps_b", tag="bank")
                nc.tensor.matmul(
                    ps[:], ones1[:], bout_row[:, c2 * 512:(c2 + 1) * 512],
                    start=True, stop=True,
                )
                nc.scalar.activation(
                    bout_bc[:, c2 * 512:(c2 + 1) * 512], ps[:], AF.Copy
                )

            gT_sb = [pgT.tile([P, T], BF16, name=f"gT{h}") for h in range(HC)]
            dT_sb = pmisc.tile([P, B * 4], F32, name="dT_sb")
            dtmp_sb = pmisc.tile([1, S_LOC], F32, name="dtmp_sb")
            dacc_sb = pmisc.tile([1, S_LOC], F32, name="dacc_sb")

            # =========== phase A: g = x_local @ M  (gT layout) ==========
            with (
                tc.tile_pool(name="xTp", bufs=1) as pxT,
                tc.tile_pool(name="wp", bufs=8) as pw,
            ):
                # issue the first-half columns of M and xT before the second
                # halves so the first psum group's operands land in ~4MB
                xT_sb = [pxT.tile([P, T], F32R, name=f"xT{h}")
                         for h in range(HC)]
                m_sb = [pw.tile([P, H], F32R, name="w", tag="w")
                        for _ in range(HC)]
                for half in range(2):
                    cs = slice(half * 512, (half + 1) * 512)
                    for h in range(HC):
                        nc.sync.dma_start(
                            m_sb[h][:, cs], m_d[h * P:(h + 1) * P, cs]
                        )
                    for h in range(HC):
                        nc.sync.dma_start(
                            xT_sb[h][:, cs], xT_d[h * P:(h + 1) * P, cs]
                        )
                for tc2 in range(2):
                    for oc in range(HC):
                        ps = pp.tile([P, 512], F32, name="ps_p", tag="bank")
                        for h in range(HC):
                            nc.tensor.matmul(
                                ps[:],
                                m_sb[h][:, oc * P:(oc + 1) * P],
                                xT_sb[h][:, tc2 * 512:(tc2 + 1) * 512],
                                start=(h == 0), stop=(h == HC - 1),
                            )
                        nc.scalar.activation(
                            gT_sb[oc][:, tc2 * 512:(tc2 + 1) * 512],
                            ps[:], AF.Copy,
                        )

            # =========== phase B: attention =============================
            with (
                tc.tile_pool(name="ktp", bufs=24) as pKT,
                tc.tile_pool(name="expp", bufs=33) as pexp,
                tc.tile_pool(name="vstp", bufs=12) as pV,
                tc.tile_pool(name="attp", bufs=1) as pattn,
                tc.tile_pool(name="wop", bufs=1) as pwo,
                tc.tile_pool(name="outp", bufs=4) as pout,
                tc.tile_pool(name="rdp", bufs=2) as prd,
            ):
                attnT = [pattn.tile([P, T], F32R, name=f"attnT{h}")
                         for h in range(HC)]
                n_sb = []
                for h in range(HC):
                    w_t = pwo.tile([P, H], F32R, name=f"wo{h}")
                    nc.sync.dma_start(w_t[:], n_d[h * P:(h + 1) * P, :])
                    n_sb.append(w_t)

                for b in range(B):
                    cb = slice(b * S_LOC, (b + 1) * S_LOC)
                    # pass A: scores + exp + d + attn for h 0..511, with the
                    # attn halves accumulated directly in PSUM over all keys
                    paA = [pp.tile([P, S_LOC], F32, name=f"paA{h2}",
                                   tag="bank") for h2 in range(4)]
                    es_all = []
                    for kg in range(NKG):
                        tau0 = b * S + kg * 512
                        kts = []
                        for oc in range(HC):
                            kt_t = pKT.tile([P, 512], BF16, name="kt",
                                            tag="kt")
                            nc.sync.dma_start(
                                kt_t[:],
                                xtf_d[oc * P:(oc + 1) * P, tau0:tau0 + 512],
                            )
                            kts.append(kt_t)
                        vts = []
                        for j in range(4):
                            v_t = pV.tile([P, 512], BF16, name="vst",
                                          tag="vst")
                            nc.sync.dma_start(
                                v_t[:],
                                xf_d[tau0 + j * P: tau0 + (j + 1) * P, 0:512],
                            )
                            vts.append(v_t)
                        for j in range(4):
                            kc = kg * 4 + j
                            ps_s = pp.tile([P, S_LOC], F32, name="ps_s",
                                           tag="bank")
                            for oc in range(HC):
                                nc.tensor.matmul(
                                    ps_s[:],
                                    kts[oc][:, j * P:(j + 1) * P],
                                    gT_sb[oc][:, cb],
                                    start=(oc == 0), stop=(oc == HC - 1),
                                )
                            e_t = pexp.tile([P, S_LOC], BF16, name="exp",
                                            tag="exp")
                            nc.scalar.activation(
                                e_t[:], ps_s[:], AF.Exp, scale=SCALE,
                                bias=c8_sb[:, b * 32 + kc: b * 32 + kc + 1],
                            )
                            es_all.append(e_t)
                            if kc == 0:
                                nc.gpsimd.tensor_reduce(
                                    dacc_sb[:], e_t[:],
                                    axis=mybir.AxisListType.C, op=ALU.add,
                                )
                            else:
                                nc.gpsimd.tensor_reduce(
                                    dtmp_sb[:], e_t[:],
                                    axis=mybir.AxisListType.C, op=ALU.add,
                                )
                                nc.gpsimd.tensor_add(
                                    dacc_sb[:], dtmp_sb[:], dacc_sb[:]
                                )
                            for h2 in range(4):
                                nc.tensor.matmul(
                                    paA[h2][:],
                                    vts[j][:, h2 * P:(h2 + 1) * P],
                                    e_t[:],
                                    start=(kc == 0), stop=(kc == NKC - 1),
                                )
                    for h2 in range(4):
                        nc.scalar.activation(attnT[h2][:, cb], paA[h2][:],
                                             AF.Copy)

                    # pass B: attn for h 512..1023, re-reading the exp tiles
                    paB = [pp.tile([P, S_LOC], F32, name=f"paB{h2}",
                                   tag="bank") for h2 in range(4)]
                    for kg in range(NKG):
                        tau0 = b * S + kg * 512
                        for j in range(4):
                            kc = kg * 4 + j
                            v_t = pV.tile([P, 512], BF16, name="vst",
                                          tag="vst")
                            nc.sync.dma_start(
                                v_t[:],
                                xf_d[tau0 + j * P: tau0 + (j + 1) * P,
                                     512:1024],
                            )
                            for h2 in range(4):
                                nc.tensor.matmul(
                                    paB[h2][:],
                                    v_t[:, h2 * P:(h2 + 1) * P],
                                    es_all[kc][:],
                                    start=(kc == 0), stop=(kc == NKC - 1),
                                )
                    for h2 in range(4):
                        nc.scalar.activation(attnT[4 + h2][:, cb], paB[h2][:],
                                             AF.Copy)
                    d_sb = prd.tile([1, S_LOC], F32, name="d_sb", tag="rd")
                    nc.vector.reciprocal(d_sb[:], dacc_sb[:])
                    for tt in range(4):
                        nc.sync.dma_start(
                            dT_sb[:, b * 4 + tt: b * 4 + tt + 1],
                            d_sb[0:1, tt * P:(tt + 1) * P],
                        )

                    # ---- output projection for this batch's tokens ----
                    for ts in range(b * 4, b * 4 + 4):
                        for oc2 in range(2):
                            po = pp.tile([P, 512], F32, name="po", tag="bank")
                            for h2 in range(HC):
                                nc.tensor.matmul(
                                    po[:],
                                    attnT[h2][:, ts * P:(ts + 1) * P],
                                    n_sb[h2][:, oc2 * 512:(oc2 + 1) * 512],
                                    start=(h2 == 0), stop=(h2 == HC - 1),
                                )
                            o_t = pout.tile([P, 512], F32, name="ot", tag="ot")
                            nc.vector.scalar_tensor_tensor(
                                o_t[:], po[:], dT_sb[:, ts:ts + 1],
                                bout_bc[:, oc2 * 512:(oc2 + 1) * 512],
                                ALU.mult, ALU.add,
                            )
                            nc.sync.dma_start(
                                out_d[ts * P:(ts + 1) * P,
                                      oc2 * 512:(oc2 + 1) * 512],
                                o_t[:],
                            )

    nc.compile()
    return nc


def make_in_maps(x, Wq, bq, Wk, bk, Wv, bv, Wo, bo):
    x = np.asarray(x, np.float32)
    Wq = np.asarray(Wq, np.float32)
    Wk = np.asarray(Wk, np.float32)
    Wv = np.asarray(Wv, np.float32)
    Wo = np.asarray(Wo, np.float32)
    bq = np.asarray(bq, np.float32)
    bk = np.asarray(bk, np.float32)
    bv = np.asarray(bv, np.float32)
    bo = np.asarray(bo, np.float32)

    m_mat = np.ascontiguousarray((Wq.T @ Wk))                 # [h_in, h_in2]
    n_mat = np.ascontiguousarray(Wv.T @ Wo.T)                 # [h_in, o]
    xfull = x.reshape(B * S, H)                               # tau-major
    xf = xfull.astype(ml_dtypes.bfloat16)
    xtf = np.ascontiguousarray(xfull.T).astype(ml_dtypes.bfloat16)
    c8 = (xfull @ (Wk.T @ bq)) * np.float32(SCALE)            # key-side bias
    b_out = Wo @ bv + bo

    common = dict(m_mat=m_mat, n_mat=n_mat, xtf=xtf, xf=xf,
                  c8=c8.astype(np.float32), b_out=b_out.astype(np.float32))
    in_maps = []
    for r in range(N_CORES):
        xr = x[:, r * S_LOC:(r + 1) * S_LOC, :].reshape(T, H)
        in_maps.append(dict(xT=np.ascontiguousarray(xr.T), **common))
    return in_maps


def assemble(results):
    shards = np.stack([res["out"] for res in results])      # [R, T, H]
    return np.ascontiguousarray(
        shards.reshape(N_CORES, B, S_LOC, H).transpose(1, 0, 2, 3)
        .reshape(B, S, H)
    )


def kernel(x, Wq, bq, Wk, bk, Wv, bv, Wo, bo):
    if "nc" not in _CACHE:
        _CACHE["nc"] = build_program()
    nc = _CACHE["nc"]
    in_maps = make_in_maps(x, Wq, bq, Wk, bk, Wv, bv, Wo, bo)
    res = run_bass_kernel_spmd(nc, in_maps, core_ids=list(range(N_CORES)))
    return assemble(res.results)
